# revision 46
# baseline (speedup 1.0000x reference)
"""Trainium2 Bass kernel for nn_MaxCDFdp_multiclass.

Computes max over (class, probe) of |ECDF0 - ECDF1| where the ECDFs are
sigmoid-smoothed empirical CDFs of y_pred per class, for the two groups
defined by s in {0,1}.

v4: narrow-window evaluation with fp16 operands, ACT-bound pipeline.
sigmoid(10*(grid - y)) is within ~1e-2 of {0,1} outside |grid - y| <=
MARGIN=0.45, and the resulting one-sided per-cell bias (~0.04*sigma(-4.5))
cancels between the two groups, so only W=14 of the 100 probes need
evaluation per (sample, class) tile. Host sorts each group per class,
cuts sorted samples into tiles of <=128 rows whose per-class span fits
a k*W-probe window (k=1 in the bulk; distribution tails widen to k=2..4
and are emitted as k standard device sub-tiles over the same samples
with A shifted by W*step*j - identical sigmoid values to a wide
window, no extra approximation). Window base B per (tile, class):
probes >= B+k*W-1 are treated as saturated (the last sub-tile's last
column is added to all later probes on host); probes < B are dropped.

Device, per group of tiles (group sizes ramp 1,2,4,8,12 then G=16,
sized so each ADD hides under the prior group's sigmoid; all
elementwise operands fp16 with the diff layout [w-outer, c-inner] so
every tensor_tensor operand has a packed 2-byte innermost dim, enabling
the DVE 2x mode):
  DVE: diff[128, g, W, C] = Dj_bcast + A_bcast     (one op, ~0.56ns/col)
  ACT: sig = sigmoid(10*diff) -> fp16              (pacing engine,
       0.834 ns/col + 185ns/instr; the single act-table load hoists to
       t~6.5us because no other ACT-engine op precedes it)
  PE : acc[chunk, 2] = sig_chunk.T @ [ind_t0, ind_t0+1]: the group's
       flat gn*280 sig columns are cut into full-128-col stationary
       loads that may span two adjacent tiles (partial-width LDWEIGHTS
       measured ~5x slower per col than full-width); the host picks the
       valid output column per row. Tiles are single-group, so one
       indicator column per tile; host routes sums to the tile's group.
Drains: two DVE copies at the end of the Vector stream; the first
(tiles < t_d) only waits on PE through t_d so it overlaps the last
groups' sigmoids. Exactly 3 input DMAs + 2 output DMAs, all on the SP
sequencer: a 4th+ concurrent DMA stalls ~4.5us on the DGE ring, and
per-engine DMA turnaround is ~2.8us (observed).
Host: relocate each tile's [W, C] window into [C, P] of its group at
its B offsets (+ saturated tail), sum over cores, divide by group
counts, abs, max.

Measured ~29.4us HW exec on 8 cores (from the 82.1us baseline), rel
err ~2.7e-4 vs the reference (tolerance 2e-2). Fixed costs: ~6.5us
queue preamble + ~1.3us act-table load + ~2.2us DMA chain before the
first sigmoid, and ~8.4us post-work semaphore drain, all
instruction-count-independent. NOTE: the device occasionally runs at
~1.0GHz instead of 1.2GHz (ACT_TABLE_LOAD slice shows 1539ns instead
of 1283ns in the trace) - compare timings only between runs with equal
table-load duration.
"""

import os
from contextlib import ExitStack

import numpy as np

import concourse.bass as bass
import concourse.bacc as bacc
import concourse.tile as tile
from concourse import mybir
from concourse.bass_utils import run_bass_kernel_spmd

N, C, P = 50000, 20, 100
TEMP = 10.0
NCORES = 8
PART = 128
W = 14                 # probe-window width per tile
CW = C * W             # 280
G = 16                 # max tiles per device group
MARGIN = 0.45          # |grid - y| cutoff (4.5 in sigmoid-arg units)

_F32 = mybir.dt.float32
_F16 = mybir.dt.float16

_CACHED = {}


def _chunks(groups):
    """Reduction chunks: each group's flat gn*CW sig columns cut into
    <=128-col stationary loads that may span two adjacent tiles. Chunk
    columns multiply the 2-col moving indicator [ind_t0, ind_t0+1]; the
    host selects the correct output column per row. Returns
    [(g0, qs, qe, t0)] in emission order; psum/output column of chunk k
    is 2k:2k+2."""
    out = []
    for g0, gn in groups:
        ncols = gn * CW
        for qs in range(0, ncols, 128):
            qe = min(qs + 128, ncols)
            out.append((g0, qs, qe, (g0 * CW + qs) // CW))
    return out


def _groups(T):
    # small first group (ACT can start right after its table load) and
    # small-ish last group (short PE drain tail)
    # ramp up so the pipeline fills while the input DMAs land, big G-tile
    # groups in the middle to amortize per-instruction overhead, taper at
    # the end so the PE/drain tail after the last sigmoid stays short
    groups = []
    i = 0
    for n in (1, 2, 4, 8, 12):
        if n >= G or i >= T:
            break
        n = min(n, T - i)
        groups.append((i, n))
        i += n
    while T - i > 0:
        groups.append((i, min(G, T - i)))
        i += min(G, T - i)
    return groups


def _split_tile(T, groups):
    # input DMA A-spans: b1 = first group only (smallest possible first
    # transfer), b2 = next few groups, b3 = the rest; t_d = accumulator
    # split before the last two groups for an early drain
    n1 = min(4, len(groups))
    t1 = groups[n1 - 1][0] + groups[n1 - 1][1]
    n2 = min(n1 + 2, len(groups))
    t2 = groups[n2 - 1][0] + groups[n2 - 1][1]
    n3 = max(0, len(groups) - 1)
    t_d = groups[n3][0] if n3 < len(groups) else T
    return t1, t2, t_d


def _build_bass(T):
    # blob free-dim layout: [Dj: W*C][ind: T][A: T*C], all fp16.
    # Exactly three input DMAs (a 4th+ concurrent dma_start stalls
    # ~4.5us on the DGE credit ring): b1 = dj+ind+A[:t1] (everything
    # the ramp groups touch), b2/b3 = the remaining A in two spans.
    dw, iw, aw = CW, T + 1, T * C
    blob_w = dw + iw + aw
    nc = bacc.Bacc(None, target_bir_lowering=False)

    groups = _groups(T)
    t1, t2, t_d = _split_tile(T, groups)
    chunks = _chunks(groups)
    n_ch1 = sum(1 for g0, _, _, _ in chunks if g0 < t_d)
    ow = 2 * len(chunks)

    b_d = nc.dram_tensor("b", [PART, blob_w], _F16, kind="ExternalInput")
    o_d = nc.dram_tensor("o", [PART, ow], _F32, kind="ExternalOutput")

    with ExitStack() as ctx:
        tc = ctx.enter_context(tile.TileContext(nc))
        constp = ctx.enter_context(tc.tile_pool(name="const", bufs=1))
        diffp = ctx.enter_context(tc.tile_pool(name="diff", bufs=3))
        sigp = ctx.enter_context(tc.tile_pool(name="sig", bufs=4))
        psump = ctx.enter_context(
            tc.tile_pool(name="psum", bufs=1, space=bass.MemorySpace.PSUM)
        )
        outp = ctx.enter_context(tc.tile_pool(name="outp", bufs=1))

        s1 = dw + iw + t1 * C
        s2 = dw + iw + t2 * C
        b1 = constp.tile([PART, s1], _F16)
        nc.sync.dma_start(b1[:], b_d[:, 0:s1])
        b2 = constp.tile([PART, s2 - s1], _F16)
        nc.sync.dma_start(b2[:], b_d[:, s1:s2])
        b3 = constp.tile([PART, blob_w - s2], _F16)
        nc.sync.dma_start(b3[:], b_d[:, s2:])
        dj_sb = b1[:, 0:dw].rearrange("p (w c) -> p w c", w=W)
        ind_sb = b1[:, dw : dw + iw]  # [128, T+1], last column zero pad
        a_srcs = [
            (0, t1, b1[:, dw + iw :].rearrange("p (t c) -> p t c", c=C)),
            (t1, t2, b2[:].rearrange("p (t c) -> p t c", c=C)),
            (t2, T, b3[:].rearrange("p (t c) -> p t c", c=C)),
        ]

        # chunk k's reduction lands at psum cols 2k:2k+2; rows = position
        # within the chunk. acc1 (groups < t_d) drains early, under the
        # last groups' compute.
        acc1 = psump.tile([PART, 2 * n_ch1], _F32)
        acc2 = psump.tile([PART, ow - 2 * n_ch1], _F32)

        def acc_slice(k):
            col = 2 * k
            if k < n_ch1:
                return acc1[:, col : col + 2]
            col -= 2 * n_ch1
            return acc2[:, col : col + 2]

        out1 = outp.tile([PART, 2 * n_ch1], _F32)
        out2 = outp.tile([PART, ow - 2 * n_ch1], _F32)

        k = 0
        for g0, gn in groups:
            diff = diffp.tile([PART, G, W, C], _F16, tag="diff")
            dj_v = dj_sb.unsqueeze(1).broadcast_to([PART, gn, W, C])
            a_sb, off = next(
                (src, lo) for lo, hi, src in a_srcs if lo <= g0 < hi
            )
            a_v = (
                a_sb[:, g0 - off : g0 - off + gn, :]
                .unsqueeze(2)
                .broadcast_to([PART, gn, W, C])
            )
            nc.vector.tensor_add(diff[:, 0:gn], dj_v, a_v)

            sig = sigp.tile([PART, G, W, C], _F16, tag="sig")
            nc.scalar.activation(
                sig[:, 0:gn], diff[:, 0:gn],
                mybir.ActivationFunctionType.Sigmoid, scale=TEMP,
            )
            sig_f = sig[:].rearrange("p t w c -> p (t w c)")

            ncols = gn * CW
            for qs in range(0, ncols, 128):
                qe = min(qs + 128, ncols)
                t0 = (g0 * CW + qs) // CW
                nc.tensor.matmul(
                    acc_slice(k)[0 : qe - qs, :],
                    sig_f[:, qs:qe],
                    ind_sb[:, t0 : t0 + 2],
                    start=True,
                    stop=True,
                )
                k += 1

        # drains sit after all ADDs in the in-order Vector queue; the big
        # acc1 copy only waits on PE through tile t_d, so it overlaps the
        # last two groups' sigmoids instead of serializing after them
        nc.vector.tensor_copy(out1[:], acc1[:])
        nc.sync.dma_start(o_d[:, 0 : 2 * n_ch1], out1[:])
        nc.vector.tensor_copy(out2[:], acc2[:])
        nc.sync.dma_start(o_d[:, 2 * n_ch1 :], out2[:])

    nc.finalize()
    return nc


def _get_nc(T):
    if T not in _CACHED:
        _CACHED[T] = _build_bass(T)
    return _CACHED[T]


# test.py reads this after calling kernel() for profiling info
LAST_RESULTS = None
LAST_DELTA = None


def kernel(y_pred: np.ndarray, s: np.ndarray) -> np.ndarray:
    global LAST_RESULTS
    y = np.ascontiguousarray(np.asarray(y_pred), dtype=np.float32)
    s_np = np.asarray(s)
    assert y.shape == (N, C)

    mn = y.min(axis=0)
    mx = y.max(axis=0)
    step = (mx.astype(np.float64) - mn) / (P - 1)  # f64 for window math

    srt0 = np.sort(y[s_np == 0], axis=0)  # [n0, C], per-class sorted
    srt1 = np.sort(y[s_np == 1], axis=0)
    n0, n1 = srt0.shape[0], srt1.shape[0]

    # global sub-tile list over both groups: (gi, vals[cnt, C], Bsub[C],
    # tail). Distribution tails, where a 128-row span exceeds the W-probe
    # budget, use a k*W-wide window emitted as k device sub-tiles over the
    # SAME samples (A shifted by W*step per sub-tile, identical sigmoid
    # values to a wide window); only the last sub-tile (tail=True) feeds
    # the saturated-probes add.
    tiles = []
    for gi, blk in enumerate((srt0, srt1)):
        m = blk.shape[0]
        start = 0
        while start < m:
            end0 = min(start + PART, m)
            for k in (1, 2, 3, 4):
                smaxk = (W * k - 2) * step - 2 * MARGIN
                lim = m
                for c in range(C):
                    e = np.searchsorted(
                        blk[:, c], blk[start, c] + smaxk[c], "right"
                    )
                    lim = min(lim, e)
                if lim >= end0 or k == 4:
                    break
            end = min(end0, max(lim, start + 1))
            vals = blk[start:end]
            ymax_t = vals.max(axis=0).astype(np.float64)
            B = (
                np.ceil((ymax_t + MARGIN - mn) / step).astype(np.int64)
                - W * k + 1
            )
            B = np.clip(B, 0, P - W * k)
            for j in range(k):
                tiles.append((gi, vals, B + W * j, j == k - 1))
            start = end

    # deal round-robin across cores so cut tiles spread evenly
    core_tiles = [tiles[r::NCORES] for r in range(NCORES)]
    T = max(len(t) for t in core_tiles)

    jj = np.arange(W, dtype=np.float32)
    dj = (step.astype(np.float32)[:, None] * jj[None, :]).astype(np.float32)
    dj_wc = np.ascontiguousarray(dj.T)  # [W, C]

    dw, iw, aw = CW, T + 1, T * C
    in_maps = []
    b_tabs = []
    for r in range(NCORES):
        ctiles = core_tiles[r]
        A = np.zeros((PART, T, C), np.float16)
        ind = np.zeros((PART, T + 1, 1), np.float16)
        Btab = np.zeros((T, C), np.int32)
        for t, (gi, vals, B, tail) in enumerate(ctiles):
            cnt = vals.shape[0]
            Btab[t] = B
            base = (mn + step * B).astype(np.float32)  # [C]
            A[:cnt, t, :] = (base[None, :] - vals).astype(np.float16)
            A[cnt:, t, :] = (base[None, :] - vals[-1]).astype(np.float16)
            ind[:cnt, t, 0] = 1.0  # tile is single-group; host adds into gi
        blob = np.empty((PART, blob_w := dw + iw + aw), np.float16)
        blob[:, 0:dw] = np.broadcast_to(dj_wc.reshape(1, dw), (PART, dw))
        blob[:, dw : dw + iw] = ind.reshape(PART, iw)
        blob[:, dw + iw :] = A.reshape(PART, aw)
        in_maps.append({"b": blob})
        b_tabs.append(Btab)

    nc = _get_nc(T)
    res = run_bass_kernel_spmd(
        nc,
        in_maps,
        core_ids=list(range(NCORES)),
        trace=bool(int(os.environ.get("BASS_KERNEL_TRACE", "0"))),
    )
    LAST_RESULTS = res

    groups = _groups(T)
    chunks = _chunks(groups)
    full = np.zeros((2, C, P + W), np.float32)  # halo simplifies the tail add
    for r in range(NCORES):
        o = res.results[r]["o"]  # [128, 2*len(chunks)]
        # chunk k covers flat cols [g0*CW+qs, g0*CW+qe); rows belonging to
        # tile t0 use output column 2k, rows of tile t0+1 use 2k+1
        flat = np.empty(T * CW, np.float32)
        for k, (g0, qs, qe, t0) in enumerate(chunks):
            f0 = g0 * CW + qs
            rows = qe - qs
            sel = (np.arange(f0, f0 + rows) // CW) == t0
            flat[f0 : f0 + rows] = np.where(sel, o[0:rows, 2 * k], o[0:rows, 2 * k + 1])
        arr = flat.reshape(T, W, C)
        Btab = b_tabs[r]
        for t, (gi, _, _, tail) in enumerate(core_tiles[r]):
            for c in range(C):
                B = Btab[t, c]
                full[gi, c, B : B + W] += arr[t, :, c]
                if tail:
                    full[gi, c, B + W :] += arr[t, W - 1, c]
    full = full[:, :, :P]
    delta = np.abs(full[0] / np.float32(n0) - full[1] / np.float32(n1))
    global LAST_DELTA
    LAST_DELTA = delta
    return np.array(delta.max(), dtype=np.float32)


# revision 47
# speedup vs baseline: 1.0566x; 1.0566x over previous
"""Trainium2 Bass kernel for nn_MaxCDFdp_multiclass.

Computes max over (class, probe) of |ECDF0 - ECDF1| where the ECDFs are
sigmoid-smoothed empirical CDFs of y_pred per class, for the two groups
defined by s in {0,1}.

v4: narrow-window evaluation with fp16 operands, ACT-bound pipeline.
sigmoid(10*(grid - y)) is within ~1e-2 of {0,1} outside |grid - y| <=
MARGIN=0.45, and the resulting one-sided per-cell bias (~0.04*sigma(-4.5))
cancels between the two groups, so only W=14 of the 100 probes need
evaluation per (sample, class) tile. Host sorts each group per class,
cuts sorted samples into tiles of <=128 rows whose per-class span fits
a k*W-probe window (k=1 in the bulk; distribution tails widen to k=2..4
and are emitted as k standard device sub-tiles over the same samples
with A shifted by W*step*j - identical sigmoid values to a wide
window, no extra approximation). Window base B per (tile, class):
probes >= B+k*W-1 are treated as saturated (the last sub-tile's last
column is added to all later probes on host); probes < B are dropped.

Device, per group of tiles (group sizes ramp 1,2,4,8,12 then G=16,
sized so each ADD hides under the prior group's sigmoid; all
elementwise operands fp16 with the diff layout [w-outer, c-inner] so
every tensor_tensor operand has a packed 2-byte innermost dim, enabling
the DVE 2x mode):
  DVE: diff[128, g, W, C] = Dj_bcast + A_bcast     (one op, ~0.56ns/col)
  ACT: sig = sigmoid(10*diff) -> fp16              (pacing engine,
       0.834 ns/col + 185ns/instr; the single act-table load hoists to
       t~6.5us because no other ACT-engine op precedes it)
  PE : acc[chunk, 2] = sig_chunk.T @ [ind_t0, ind_t0+1]: the group's
       flat gn*280 sig columns are cut into full-128-col stationary
       loads that may span two adjacent tiles (partial-width LDWEIGHTS
       measured ~5x slower per col than full-width); the host picks the
       valid output column per row. Tiles are single-group, so one
       indicator column per tile; host routes sums to the tile's group.
Drains: two DVE copies at the end of the Vector stream; the first
(tiles < t_d) only waits on PE through t_d so it overlaps the last
groups' sigmoids. Exactly 3 input DMAs + 2 output DMAs, all on the SP
sequencer: a 4th+ concurrent DMA stalls ~4.5us on the DGE ring, and
per-engine DMA turnaround is ~2.8us (observed).
Host: relocate each tile's [W, C] window into [C, P] of its group at
its B offsets (+ saturated tail), sum over cores, divide by group
counts, abs, max.

Measured ~29.4us HW exec on 8 cores (from the 82.1us baseline), rel
err ~2.7e-4 vs the reference (tolerance 2e-2). Fixed costs: ~6.5us
queue preamble + ~1.3us act-table load + ~2.2us DMA chain before the
first sigmoid, and ~8.4us post-work semaphore drain, all
instruction-count-independent. NOTE: the device occasionally runs at
~1.0GHz instead of 1.2GHz (ACT_TABLE_LOAD slice shows 1539ns instead
of 1283ns in the trace) - compare timings only between runs with equal
table-load duration.
"""

import os
from contextlib import ExitStack

import numpy as np

import concourse.bass as bass
import concourse.bacc as bacc
import concourse.tile as tile
from concourse import mybir
from concourse.bass_utils import run_bass_kernel_spmd

N, C, P = 50000, 20, 100
TEMP = 10.0
NCORES = 8
PART = 128
W = 13                 # probe-window width per tile
CW = C * W             # 260
G = 16                 # max tiles per device group
MARGIN = 0.40          # |grid - y| cutoff (4.0 in sigmoid-arg units)

_F32 = mybir.dt.float32
_F16 = mybir.dt.float16

_CACHED = {}


def _chunks(groups):
    """Reduction chunks: each group's flat gn*CW sig columns cut into
    <=128-col stationary loads that may span two adjacent tiles. Chunk
    columns multiply the 2-col moving indicator [ind_t0, ind_t0+1]; the
    host selects the correct output column per row. Returns
    [(g0, qs, qe, t0)] in emission order; psum/output column of chunk k
    is 2k:2k+2."""
    out = []
    for g0, gn in groups:
        ncols = gn * CW
        for qs in range(0, ncols, 128):
            qe = min(qs + 128, ncols)
            out.append((g0, qs, qe, (g0 * CW + qs) // CW))
    return out


def _groups(T):
    # small first group (ACT can start right after its table load) and
    # small-ish last group (short PE drain tail)
    # ramp up so the pipeline fills while the input DMAs land, big G-tile
    # groups in the middle to amortize per-instruction overhead, taper at
    # the end so the PE/drain tail after the last sigmoid stays short
    groups = []
    i = 0
    for n in (1, 2, 4, 8, 12):
        if n >= G or i >= T:
            break
        n = min(n, T - i)
        groups.append((i, n))
        i += n
    while T - i > 0:
        groups.append((i, min(G, T - i)))
        i += min(G, T - i)
    return groups


def _split_tile(T, groups):
    # input DMA A-spans: b1 = first group only (smallest possible first
    # transfer), b2 = next few groups, b3 = the rest; t_d = accumulator
    # split before the last two groups for an early drain
    n1 = min(4, len(groups))
    t1 = groups[n1 - 1][0] + groups[n1 - 1][1]
    n2 = min(n1 + 2, len(groups))
    t2 = groups[n2 - 1][0] + groups[n2 - 1][1]
    n3 = max(0, len(groups) - 1)
    t_d = groups[n3][0] if n3 < len(groups) else T
    return t1, t2, t_d


def _build_bass(T):
    # blob free-dim layout: [Dj: W*C][ind: T][A: T*C], all fp16.
    # Exactly three input DMAs (a 4th+ concurrent dma_start stalls
    # ~4.5us on the DGE credit ring): b1 = dj+ind+A[:t1] (everything
    # the ramp groups touch), b2/b3 = the remaining A in two spans.
    dw, iw, aw = CW, T + 1, T * C
    blob_w = dw + iw + aw
    nc = bacc.Bacc(None, target_bir_lowering=False)

    groups = _groups(T)
    t1, t2, t_d = _split_tile(T, groups)
    chunks = _chunks(groups)
    n_ch1 = sum(1 for g0, _, _, _ in chunks if g0 < t_d)
    ow = 2 * len(chunks)

    b_d = nc.dram_tensor("b", [PART, blob_w], _F16, kind="ExternalInput")
    o_d = nc.dram_tensor("o", [PART, ow], _F32, kind="ExternalOutput")

    with ExitStack() as ctx:
        tc = ctx.enter_context(tile.TileContext(nc))
        constp = ctx.enter_context(tc.tile_pool(name="const", bufs=1))
        diffp = ctx.enter_context(tc.tile_pool(name="diff", bufs=3))
        sigp = ctx.enter_context(tc.tile_pool(name="sig", bufs=4))
        psump = ctx.enter_context(
            tc.tile_pool(name="psum", bufs=1, space=bass.MemorySpace.PSUM)
        )
        outp = ctx.enter_context(tc.tile_pool(name="outp", bufs=1))

        s1 = dw + iw + t1 * C
        s2 = dw + iw + t2 * C
        b1 = constp.tile([PART, s1], _F16)
        nc.sync.dma_start(b1[:], b_d[:, 0:s1])
        b2 = constp.tile([PART, s2 - s1], _F16)
        nc.sync.dma_start(b2[:], b_d[:, s1:s2])
        b3 = constp.tile([PART, blob_w - s2], _F16)
        nc.sync.dma_start(b3[:], b_d[:, s2:])
        dj_sb = b1[:, 0:dw].rearrange("p (w c) -> p w c", w=W)
        ind_sb = b1[:, dw : dw + iw]  # [128, T+1], last column zero pad
        a_srcs = [
            (0, t1, b1[:, dw + iw :].rearrange("p (t c) -> p t c", c=C)),
            (t1, t2, b2[:].rearrange("p (t c) -> p t c", c=C)),
            (t2, T, b3[:].rearrange("p (t c) -> p t c", c=C)),
        ]

        # chunk k's reduction lands at psum cols 2k:2k+2; rows = position
        # within the chunk. acc1 (groups < t_d) drains early, under the
        # last groups' compute.
        acc1 = psump.tile([PART, 2 * n_ch1], _F32)
        acc2 = psump.tile([PART, ow - 2 * n_ch1], _F32)

        def acc_slice(k):
            col = 2 * k
            if k < n_ch1:
                return acc1[:, col : col + 2]
            col -= 2 * n_ch1
            return acc2[:, col : col + 2]

        out1 = outp.tile([PART, 2 * n_ch1], _F32)
        out2 = outp.tile([PART, ow - 2 * n_ch1], _F32)

        k = 0
        for g0, gn in groups:
            diff = diffp.tile([PART, G, W, C], _F16, tag="diff")
            dj_v = dj_sb.unsqueeze(1).broadcast_to([PART, gn, W, C])
            a_sb, off = next(
                (src, lo) for lo, hi, src in a_srcs if lo <= g0 < hi
            )
            a_v = (
                a_sb[:, g0 - off : g0 - off + gn, :]
                .unsqueeze(2)
                .broadcast_to([PART, gn, W, C])
            )
            nc.vector.tensor_add(diff[:, 0:gn], dj_v, a_v)

            sig = sigp.tile([PART, G, W, C], _F16, tag="sig")
            nc.scalar.activation(
                sig[:, 0:gn], diff[:, 0:gn],
                mybir.ActivationFunctionType.Sigmoid, scale=TEMP,
            )
            sig_f = sig[:].rearrange("p t w c -> p (t w c)")

            ncols = gn * CW
            for qs in range(0, ncols, 128):
                qe = min(qs + 128, ncols)
                t0 = (g0 * CW + qs) // CW
                nc.tensor.matmul(
                    acc_slice(k)[0 : qe - qs, :],
                    sig_f[:, qs:qe],
                    ind_sb[:, t0 : t0 + 2],
                    start=True,
                    stop=True,
                )
                k += 1

        # drains sit after all ADDs in the in-order Vector queue; the big
        # acc1 copy only waits on PE through tile t_d, so it overlaps the
        # last two groups' sigmoids instead of serializing after them
        nc.vector.tensor_copy(out1[:], acc1[:])
        nc.sync.dma_start(o_d[:, 0 : 2 * n_ch1], out1[:])
        nc.vector.tensor_copy(out2[:], acc2[:])
        nc.sync.dma_start(o_d[:, 2 * n_ch1 :], out2[:])

    nc.finalize()
    return nc


def _get_nc(T):
    if T not in _CACHED:
        _CACHED[T] = _build_bass(T)
    return _CACHED[T]


# test.py reads this after calling kernel() for profiling info
LAST_RESULTS = None
LAST_DELTA = None


def kernel(y_pred: np.ndarray, s: np.ndarray) -> np.ndarray:
    global LAST_RESULTS
    y = np.ascontiguousarray(np.asarray(y_pred), dtype=np.float32)
    s_np = np.asarray(s)
    assert y.shape == (N, C)

    mn = y.min(axis=0)
    mx = y.max(axis=0)
    step = (mx.astype(np.float64) - mn) / (P - 1)  # f64 for window math

    srt0 = np.sort(y[s_np == 0], axis=0)  # [n0, C], per-class sorted
    srt1 = np.sort(y[s_np == 1], axis=0)
    n0, n1 = srt0.shape[0], srt1.shape[0]

    # global sub-tile list over both groups: (gi, vals[cnt, C], Bsub[C],
    # tail). Distribution tails, where a 128-row span exceeds the W-probe
    # budget, use a k*W-wide window emitted as k device sub-tiles over the
    # SAME samples (A shifted by W*step per sub-tile, identical sigmoid
    # values to a wide window); only the last sub-tile (tail=True) feeds
    # the saturated-probes add.
    tiles = []
    for gi, blk in enumerate((srt0, srt1)):
        m = blk.shape[0]
        start = 0
        while start < m:
            end0 = min(start + PART, m)
            for k in (1, 2, 3, 4):
                smaxk = (W * k - 2) * step - 2 * MARGIN
                lim = m
                for c in range(C):
                    e = np.searchsorted(
                        blk[:, c], blk[start, c] + smaxk[c], "right"
                    )
                    lim = min(lim, e)
                if lim >= end0 or k == 4:
                    break
            end = min(end0, max(lim, start + 1))
            vals = blk[start:end]
            ymax_t = vals.max(axis=0).astype(np.float64)
            B = (
                np.ceil((ymax_t + MARGIN - mn) / step).astype(np.int64)
                - W * k + 1
            )
            B = np.clip(B, 0, P - W * k)
            for j in range(k):
                tiles.append((gi, vals, B + W * j, j == k - 1))
            start = end

    # deal round-robin across cores so cut tiles spread evenly
    core_tiles = [tiles[r::NCORES] for r in range(NCORES)]
    T = max(len(t) for t in core_tiles)

    jj = np.arange(W, dtype=np.float32)
    dj = (step.astype(np.float32)[:, None] * jj[None, :]).astype(np.float32)
    dj_wc = np.ascontiguousarray(dj.T)  # [W, C]

    dw, iw, aw = CW, T + 1, T * C
    in_maps = []
    b_tabs = []
    for r in range(NCORES):
        ctiles = core_tiles[r]
        A = np.zeros((PART, T, C), np.float16)
        ind = np.zeros((PART, T + 1, 1), np.float16)
        Btab = np.zeros((T, C), np.int32)
        for t, (gi, vals, B, tail) in enumerate(ctiles):
            cnt = vals.shape[0]
            Btab[t] = B
            base = (mn + step * B).astype(np.float32)  # [C]
            A[:cnt, t, :] = (base[None, :] - vals).astype(np.float16)
            A[cnt:, t, :] = (base[None, :] - vals[-1]).astype(np.float16)
            ind[:cnt, t, 0] = 1.0  # tile is single-group; host adds into gi
        blob = np.empty((PART, blob_w := dw + iw + aw), np.float16)
        blob[:, 0:dw] = np.broadcast_to(dj_wc.reshape(1, dw), (PART, dw))
        blob[:, dw : dw + iw] = ind.reshape(PART, iw)
        blob[:, dw + iw :] = A.reshape(PART, aw)
        in_maps.append({"b": blob})
        b_tabs.append(Btab)

    nc = _get_nc(T)
    res = run_bass_kernel_spmd(
        nc,
        in_maps,
        core_ids=list(range(NCORES)),
        trace=bool(int(os.environ.get("BASS_KERNEL_TRACE", "0"))),
    )
    LAST_RESULTS = res

    groups = _groups(T)
    chunks = _chunks(groups)
    full = np.zeros((2, C, P + W), np.float32)  # halo simplifies the tail add
    for r in range(NCORES):
        o = res.results[r]["o"]  # [128, 2*len(chunks)]
        # chunk k covers flat cols [g0*CW+qs, g0*CW+qe); rows belonging to
        # tile t0 use output column 2k, rows of tile t0+1 use 2k+1
        flat = np.empty(T * CW, np.float32)
        for k, (g0, qs, qe, t0) in enumerate(chunks):
            f0 = g0 * CW + qs
            rows = qe - qs
            sel = (np.arange(f0, f0 + rows) // CW) == t0
            flat[f0 : f0 + rows] = np.where(sel, o[0:rows, 2 * k], o[0:rows, 2 * k + 1])
        arr = flat.reshape(T, W, C)
        Btab = b_tabs[r]
        for t, (gi, _, _, tail) in enumerate(core_tiles[r]):
            for c in range(C):
                B = Btab[t, c]
                full[gi, c, B : B + W] += arr[t, :, c]
                if tail:
                    full[gi, c, B + W :] += arr[t, W - 1, c]
    full = full[:, :, :P]
    delta = np.abs(full[0] / np.float32(n0) - full[1] / np.float32(n1))
    global LAST_DELTA
    LAST_DELTA = delta
    return np.array(delta.max(), dtype=np.float32)
